# revision 1
# baseline (speedup 1.0000x reference)
"""SSIM masked loss kernel for Trainium2 (8 NeuronCores, data-parallel over batch).

Strategy
--------
Pure data parallel: B=16 images sharded 2 per core.  Per (b, c) we need the
five 11x11 Gaussian-window maps mu1, mu2, E[x1^2]+E[x2^2], E[x1 x2] plus the
box-filtered validity mask per b.  The separable conv is done entirely on the
TensorEngine as two banded matmuls:

  pass 1 (vertical):   T = X^T @ G1     -- image slice is the *stationary*
                                           operand, so the output lands
                                           transposed ([W, H]) for free
  pass 2 (horizontal): F = G2^T @ T     -- banded Gaussian stationary

H and W are tiled in 118-wide output tiles whose 128-row input windows overlap
by 10 rows, so every output tile is a single K<=128 contraction (no halo
matmuls).  SAME zero-padding is folded into the band matrices (truncated
bands at the image edges).

The per-pixel SSIM arithmetic runs on DVE/ACT/GPSIMD reading the conv results
straight out of PSUM, and the masked reduction is fused into a
tensor_tensor_reduce plus a tiny ones-vector matmul for the cross-partition
sum.  Each core returns per-image partial sums; the final few flops run on
host in numpy.
"""

import sys

import numpy as np

sys.path.insert(0, "/opt/trn_rl_repo")

import concourse.bass as bass  # noqa: E402
import concourse.bacc as bacc  # noqa: E402
from concourse import mybir  # noqa: E402
from concourse.bass_utils import run_bass_kernel_spmd  # noqa: E402
from concourse.tile import TileContext  # noqa: E402

WS = 11
PAD = WS // 2
SIGMA = 1.5
C1 = np.float32(0.01**2)
C2 = np.float32(0.03**2)

B, C, H, W = 16, 3, 512, 512
NCORES = 8
BL = B // NCORES  # images per core

# output tiles of 118 rows; input windows of <=128 rows (clipped at edges)
# (in_start, in_size, out_start, out_size)
TILES = [
    (0, 123, 0, 118),
    (113, 128, 118, 118),
    (231, 128, 236, 118),
    (349, 128, 354, 118),
    (467, 45, 472, 40),
]
NT = len(TILES)

F32 = mybir.dt.float32
AF = mybir.ActivationFunctionType
ALU = mybir.AluOpType


def _gauss_taps() -> np.ndarray:
    x = np.arange(WS, dtype=np.float32) - (WS // 2)
    g = np.exp(-(x**2) / np.float32(2.0 * SIGMA * SIGMA)).astype(np.float32)
    return (g / g.sum()).astype(np.float32)


def _band(in0: int, k: int, o0: int, m: int, taps: np.ndarray) -> np.ndarray:
    """G[i, j] = taps[(in0+i) - (o0+j) + PAD]; zero outside the band."""
    gi = np.arange(k)[:, None] + in0
    gj = np.arange(m)[None, :] + o0
    d = gi - gj + PAD
    ok = (d >= 0) & (d < WS)
    out = np.zeros((k, m), np.float32)
    out[ok] = taps[d[ok]]
    return out


# packed weight layout: 4 variants x 512 cols of band matrices + 1 ones col
_VARS = ["g", "g2x2", "b1", "b2"]
_COL0 = [0, 118, 236, 354, 472]  # column offset of tile j within a variant


def _weight_arrays() -> dict[str, np.ndarray]:
    g = _gauss_taps()
    taps = {
        "g": g,
        "g2x2": (2.0 * g).astype(np.float32),
        "b1": np.ones(WS, np.float32),
        "b2": np.full(WS, np.float32(1.0 / (WS * WS)), np.float32),
    }
    wp = np.zeros((128, 4 * 512 + 1), np.float32)
    for vi, v in enumerate(_VARS):
        for j, (i0, k, o0, m) in enumerate(TILES):
            wp[:k, vi * 512 + _COL0[j] : vi * 512 + _COL0[j] + m] = _band(
                i0, k, o0, m, taps[v]
            )
    wp[:, -1] = 1.0  # ones column for the cross-partition reduction matmul
    return {"wpack": wp}


def _build_bass() -> bass.Bass:
    nc = bacc.Bacc()

    img1_d = nc.declare_dram_parameter("img1", [BL, C, H, W], F32, isOutput=False)
    img2_d = nc.declare_dram_parameter("img2", [BL, C, H, W], F32, isOutput=False)
    match_d = nc.declare_dram_parameter("matchf", [BL, 1, H, W], F32, isOutput=False)
    wpack_d = nc.declare_dram_parameter("wpack", [128, 4 * 512 + 1], F32, isOutput=False)
    out_d = nc.declare_dram_parameter("out", [BL, 2], F32, isOutput=True)

    from contextlib import ExitStack

    with TileContext(nc) as tc, ExitStack() as ctx:
        consts = ctx.enter_context(tc.tile_pool(name="consts", bufs=1))
        imgs = ctx.enter_context(tc.tile_pool(name="imgs", bufs=2))
        pre = ctx.enter_context(tc.tile_pool(name="pre", bufs=2))
        tsbp = ctx.enter_context(tc.tile_pool(name="tsb", bufs=2))
        pix = ctx.enter_context(tc.tile_pool(name="pix", bufs=2))
        maskp = ctx.enter_context(tc.tile_pool(name="maskp", bufs=1))
        accp = ctx.enter_context(tc.tile_pool(name="accp", bufs=2))
        outp = ctx.enter_context(tc.tile_pool(name="outp", bufs=2))
        psumT = ctx.enter_context(tc.tile_pool(name="psumT", bufs=4, space="PSUM"))
        psumF = ctx.enter_context(tc.tile_pool(name="psumF", bufs=3, space="PSUM"))
        psumR = ctx.enter_context(tc.tile_pool(name="psumR", bufs=1, space="PSUM"))

        # stage all band matrices in SBUF with one DMA
        wpack = consts.tile([128, 4 * 512 + 1], F32, tag="wpack", name="wpack")
        nc.sync.dma_start(out=wpack, in_=wpack_d[:, :])

        def wsl(var: str, j: int):
            vi = _VARS.index(var)
            i0, k, o0, m = TILES[j]
            c0 = vi * 512 + _COL0[j]
            return wpack[:k, c0 : c0 + m]

        def conv_pass1(src_tiles, var, wb):
            """vertical conv + transpose: returns PSUM tile [kw, 512] where
            kw is the width of W-window wb; column range j holds out-rows."""
            w0, kw, _, _ = TILES[wb]
            tp = psumT.tile([128, 512], F32, tag="T")
            for j, (i0, k, o0, m) in enumerate(TILES):
                nc.tensor.matmul(
                    tp[:kw, o0 : o0 + m],
                    src_tiles[j][:k, w0 : w0 + kw],
                    wsl(var, j),
                    start=True,
                    stop=True,
                )
            return tp

        def evac(idx, dst, src):
            if idx % 2 == 0:
                nc.vector.tensor_copy(dst, src)
            else:
                nc.scalar.copy(dst, src)

        for b in range(BL):
            # ---------------- mask pipeline (box conv of match) -------------
            mt = [imgs.tile([128, 512], F32, tag=f"match_{j}", name=f"match_{j}") for j in range(NT)]
            for j, (i0, k, o0, m) in enumerate(TILES):
                nc.sync.dma_start(out=mt[j][:k, :], in_=match_d[b, 0, i0 : i0 + k, :])

            mask_sb = []
            mcols = accp.tile([128, NT], F32, tag="mcols")
            nc.vector.memset(mcols, 0.0)
            for wb, (w0, kw, ow0, mw) in enumerate(TILES):
                tp = conv_pass1(mt, "b1", wb)
                tsb = tsbp.tile([128, 512], F32, tag="tsb_m")
                evac(wb, tsb[:kw, :], tp[:kw, :])
                fp = psumF.tile([128, 512], F32, tag="F")
                nc.tensor.matmul(
                    fp[:mw, :], wsl("b2", wb), tsb[:kw, :],
                    start=True, stop=True,
                )
                mk = maskp.tile([128, 512], F32, tag=f"mask_{wb}")
                nc.vector.tensor_scalar(
                    mk[:mw, :], fp[:mw, :], 0.5, 1e-7,
                    ALU.is_gt, ALU.add,
                    accum_out=mcols[:mw, wb : wb + 1],
                )
                mask_sb.append(mk)

            scols = accp.tile([128, C * NT], F32, tag="scols")
            nc.vector.memset(scols, 0.0)

            # ---------------- channels -------------------------------------
            for c in range(C):
                x1 = [imgs.tile([128, 512], F32, tag=f"x1_{j}", name=f"x1_{j}") for j in range(NT)]
                x2 = [imgs.tile([128, 512], F32, tag=f"x2_{j}", name=f"x2_{j}") for j in range(NT)]
                for j, (i0, k, o0, m) in enumerate(TILES):
                    nc.sync.dma_start(out=x1[j][:k, :], in_=img1_d[b, c, i0 : i0 + k, :])
                    nc.sync.dma_start(out=x2[j][:k, :], in_=img2_d[b, c, i0 : i0 + k, :])

                p12 = [pre.tile([128, 512], F32, tag=f"p12_{j}", name=f"p12_{j}") for j in range(NT)]
                ssq = [pre.tile([128, 512], F32, tag=f"ssq_{j}", name=f"ssq_{j}") for j in range(NT)]
                for j, (i0, k, o0, m) in enumerate(TILES):
                    s1 = pre.tile([128, 512], F32, tag="sq_a")
                    s2 = pre.tile([128, 512], F32, tag="sq_b")
                    nc.scalar.square(s1[:k, :], x1[j][:k, :])
                    nc.scalar.square(s2[:k, :], x2[j][:k, :])
                    nc.vector.tensor_mul(p12[j][:k, :], x1[j][:k, :], x2[j][:k, :])
                    nc.gpsimd.tensor_add(ssq[j][:k, :], s1[:k, :], s2[:k, :])

                for wb, (w0, kw, ow0, mw) in enumerate(TILES):
                    srcs = (x1, x2, p12, ssq)
                    tsbs = []
                    for mi, src in enumerate(srcs):
                        tp = conv_pass1(src, "g", wb)
                        tsb = tsbp.tile([128, 512], F32, tag=f"tsb_{mi}")
                        evac(mi, tsb[:kw, :], tp[:kw, :])
                        tsbs.append(tsb)
                    fm1 = psumF.tile([128, 512], F32, tag="F")
                    fm2 = psumF.tile([128, 512], F32, tag="F")
                    fr2 = psumF.tile([128, 512], F32, tag="F")
                    fs = psumF.tile([128, 512], F32, tag="F")
                    nc.tensor.matmul(fm1[:mw, :], wsl("g", wb),
                                     tsbs[0][:kw, :], start=True, stop=True)
                    nc.tensor.matmul(fm2[:mw, :], wsl("g", wb),
                                     tsbs[1][:kw, :], start=True, stop=True)
                    nc.tensor.matmul(fr2[:mw, :], wsl("g2x2", wb),
                                     tsbs[2][:kw, :], start=True, stop=True)
                    nc.tensor.matmul(fs[:mw, :], wsl("g", wb),
                                     tsbs[3][:kw, :], start=True, stop=True)

                    # ---- per-pixel SSIM ------------------------------------
                    m1 = pix.tile([128, 512], F32, tag="m1")
                    m1s = pix.tile([128, 512], F32, tag="m1s")
                    m2s = pix.tile([128, 512], F32, tag="m2s")
                    nc.scalar.copy(m1[:mw, :], fm1[:mw, :])
                    nc.scalar.square(m1s[:mw, :], fm1[:mw, :])
                    nc.scalar.square(m2s[:mw, :], fm2[:mw, :])

                    pm = pix.tile([128, 512], F32, tag="pm")
                    nc.vector.tensor_mul(pm[:mw, :], m1[:mw, :], fm2[:mw, :])
                    num1 = pix.tile([128, 512], F32, tag="num1")
                    nc.vector.tensor_scalar(
                        num1[:mw, :], pm[:mw, :], 2.0, float(C1), ALU.mult, ALU.add
                    )
                    q = pix.tile([128, 512], F32, tag="q")
                    nc.gpsimd.tensor_add(q[:mw, :], m1s[:mw, :], m2s[:mw, :])
                    den1 = pix.tile([128, 512], F32, tag="den1")
                    nc.vector.tensor_scalar_add(den1[:mw, :], q[:mw, :], float(C1))
                    num2 = pix.tile([128, 512], F32, tag="num2")
                    nc.vector.affine_then_add(
                        num2[:mw, :], num1[:mw, :], fr2[:mw, :],
                        scale=-1.0, bias=float(C1 + C2),
                    )
                    den2 = pix.tile([128, 512], F32, tag="den2")
                    nc.vector.affine_then_add(
                        den2[:mw, :], q[:mw, :], fs[:mw, :],
                        scale=-1.0, bias=float(C2),
                    )
                    num = pix.tile([128, 512], F32, tag="num")
                    nc.vector.tensor_mul(num[:mw, :], num1[:mw, :], num2[:mw, :])
                    den = pix.tile([128, 512], F32, tag="den")
                    nc.vector.tensor_mul(den[:mw, :], den1[:mw, :], den2[:mw, :])
                    rec = pix.tile([128, 512], F32, tag="rec")
                    nc.vector.reciprocal_approx_fast(out=rec[:mw, :], in_=den[:mw, :])
                    s = pix.tile([128, 512], F32, tag="s")
                    nc.vector.tensor_mul(s[:mw, :], num[:mw, :], rec[:mw, :])
                    junk = pix.tile([128, 512], F32, tag="pm", name="junk")
                    nc.vector.tensor_mul(junk[:mw, :], s[:mw, :], mask_sb[wb][:mw, :])
                    nc.vector.tensor_reduce(
                        scols[:mw, c * NT + wb : c * NT + wb + 1], junk[:mw, :],
                        mybir.AxisListType.X, ALU.add,
                    )

            # ---------------- per-image reduction --------------------------
            fin = accp.tile([128, 2], F32, tag="fin")
            nc.vector.tensor_reduce(fin[:, 0:1], scols[:, :], mybir.AxisListType.X, ALU.add)
            nc.vector.tensor_reduce(fin[:, 1:2], mcols[:, :], mybir.AxisListType.X, ALU.add)
            fr = psumR.tile([2, 1], F32, tag="fin_ps")
            nc.tensor.matmul(fr[:2, :1], fin[:, :2], wpack[:, -1:],
                             start=True, stop=True)
            osb = outp.tile([2, 1], F32, tag="osb")
            nc.scalar.copy(osb[:2, :1], fr[:2, :1])
            nc.sync.dma_start(out=out_d[b, :], in_=osb[:2, 0:1])

    nc.compile()
    return nc


_NC_CACHE: bass.Bass | None = None


def _get_nc() -> bass.Bass:
    global _NC_CACHE
    if _NC_CACHE is None:
        _NC_CACHE = _build_bass()
    return _NC_CACHE


def kernel(img1: np.ndarray, img2: np.ndarray, match: np.ndarray) -> np.ndarray:
    img1 = np.ascontiguousarray(img1, dtype=np.float32)
    img2 = np.ascontiguousarray(img2, dtype=np.float32)
    matchf = np.ascontiguousarray(match.astype(np.float32))

    weights = _weight_arrays()
    in_maps = []
    for i in range(NCORES):
        sl = slice(i * BL, (i + 1) * BL)
        m = {"img1": img1[sl], "img2": img2[sl], "matchf": matchf[sl]}
        m.update(weights)
        in_maps.append(m)

    nc = _get_nc()
    res = run_bass_kernel_spmd(nc, in_maps, list(range(NCORES))).results

    total = np.float64(0.0)
    for i in range(NCORES):
        o = np.asarray(res[i]["out"], dtype=np.float64)  # [BL, 2]
        s1 = o[:, 0]
        mb = o[:, 1]
        s_b = 3.0 * mb - s1
        norm = (H * W) / (mb + 1e-6)
        total += np.sum(s_b * norm)
    return np.float32(total / 3.0)



# revision 12
# speedup vs baseline: 1.3349x; 1.3349x over previous
"""SSIM masked loss kernel for Trainium2 (8 NeuronCores, data-parallel over batch).

v3 — bf16 end-to-end, packed PSUM/SBUF tiles, fused per-pixel math.

Strategy
--------
Pure data parallel: B=16 images sharded 2 per core.  Inputs are converted to
bf16 on the host (halves HBM traffic; the final loss is a 12.6M-pixel sum of
(1-ssim)*mask with ssim ~ 0.005, so per-pixel rounding noise is damped ~200x
in the result).

The separable 11-tap Gaussian window runs on the TensorEngine as two banded
matmuls per tile (pass 1: image-stationary, output lands transposed; pass 2:
band-stationary, N=512 streams), all in bf16.

Packing: the four pass-1 results share one 4-bank PSUM tile so a single
copy evacuates all of them; pass-2 puts mu1|mu2 in adjacent banks so one
ACT copy produces both scaled means.  Per-pixel math (per [mw,512] tile):

  e12  = sqrt2 * (mu1|mu2)                 (1 ACT copy, PSUM->SBUF bf16)
  P2   = e1*e2      = 2*mu1*mu2            (GPSIMD)
  U    = e1+e2                             (GPSIMD)
  U2   = (U/sqrt2)^2 = (mu1+mu2)^2         (ACT square)
  Q2   = U2 - P2    = mu1^2+mu2^2          (DVE tensor_tensor, 2x bf16)
  num  = (P2+C1)*(R-P2+C2) = num1*num2     (custom DVE; R = 2*E[x1x2])
  den  = (Q2+C1)*(S-Q2+C2) = den1*den2     (custom DVE; S = E[x1^2+x2^2])
  sel  = num * bmask                       (DVE tensor_tensor, 2x bf16)
  acc += sum(sel * recip1(den))            (custom DVE: seeded 1-Newton
                                            reciprocal * sel, fused reduce)

The mask is thresholded once per (b,wb) with a fused compare+count op.  The
reference's +1e-7 mask epsilons contribute ~2e-7 relative and are applied on
the host; per-image partials leave the device as [128,2] per-partition sums.
"""

import sys

import numpy as np

sys.path.insert(0, "/opt/trn_rl_repo")

import ml_dtypes  # noqa: E402

import concourse.bass as bass  # noqa: E402
import concourse.bacc as bacc  # noqa: E402
from concourse import mybir  # noqa: E402
from concourse.bass_utils import run_bass_kernel_spmd  # noqa: E402
from concourse.tile import TileContext  # noqa: E402

import concourse.dve_ops as dve_ops_mod  # noqa: E402
from concourse.dve_ops import DveOp  # noqa: E402
from concourse.dve_spec import (  # noqa: E402
    C0,
    C1 as DC1,
    AluOp as DAluOp,
    Bin,
    Spec,
    Src0,
    Src1,
    Zero,
    _has_src1,
    lower as dve_lower,
)
from concourse.dve_uop import DveOpSpec  # noqa: E402
from operator import add as _op_add  # noqa: E402

WS = 11
PAD = WS // 2
SIGMA = 1.5
SSIM_C1 = float(0.01**2)
SSIM_C2 = float(0.03**2)
SQRT2 = float(np.sqrt(2.0))
ISQRT2 = float(1.0 / np.sqrt(2.0))
# Chebyshev seed pair for the BITWISE_NOT reciprocal approximation
RECIP_S0 = -0.23549792
RECIP_S1 = 2.0017324

B, C, H, W = 16, 3, 512, 512
NCORES = 8
BL = B // NCORES  # images per core

# output tiles of 118 rows; input windows of <=128 rows (clipped at edges)
# (in_start, in_size, out_start, out_size)
TILES = [
    (0, 123, 0, 118),
    (113, 128, 118, 118),
    (231, 128, 236, 118),
    (349, 128, 354, 118),
    (467, 45, 472, 40),
]
NT = len(TILES)
# per-pixel stage groups of W-windows with equal mw, processed as one op
WB_GROUPS = [(0, 1), (2, 3), (4,)]
NG = len(WB_GROUPS)

F32 = mybir.dt.float32
BF16 = mybir.dt.bfloat16
AF = mybir.ActivationFunctionType
ALU = mybir.AluOpType
BF16NP = ml_dtypes.bfloat16


# ---------------------------------------------------------------- custom DVE
def _register_dve_op(name: str, spec: Spec) -> DveOp:
    if name in dve_ops_mod._SUB_OPCODE_FOR_NAME:
        return next(op for op in dve_ops_mod.OPS if op.name == name)
    row = max(dve_ops_mod._SUB_OPCODE_FOR_NAME.values()) + 1
    assert row < 0x20, "out of custom-DVE opcode rows"
    dve_ops_mod._SUB_OPCODE_FOR_NAME[name] = row
    shas = {}
    for ver in ("v3", "v4"):
        uops = dve_lower(spec, ver=ver)
        shas[ver] = DveOpSpec(
            name=name, opcode=row, uops=uops, rd1_en=_has_src1(spec)
        ).sha(ver)
    op = DveOp(name, spec, subdim=False, uops_sha=shas)
    dve_ops_mod.OPS.append(op)
    dve_ops_mod.CUSTOM_DVE_SPECS[name] = spec
    return op


# ssq = (x1-x2)^2 + s0*x1*x2; s0=2 gives x1^2 + x2^2 (pre-conv input prep).
# (sq(Src0)+sq(Src1) hard-crashes the exec unit; only pipeline values can
# be squared, so use the (a-b)^2+2ab form.)
SQSUM_ANT = _register_dve_op(
    "SQSUM_ANT",
    Spec(
        body=(Src0 - Src1) * (Src0 - Src1) + Src0 * Src1 * C0,
        reference=lambda in0, in1, s0, s1, imm2: (
            (in0.astype(np.float32) - in1.astype(np.float32)) ** 2
            + in0.astype(np.float32) * in1.astype(np.float32) * s0
        ),
    ),
)

# out = (in0 + s0) * (in1 - in0 + s1): num1*num2 / den1*den2 in one op
FMA2_ANT = _register_dve_op(
    "FMA2_ANT",
    Spec(
        body=(Src0 + C0) * (Src1 - Src0 + DC1),
        reference=lambda in0, in1, s0, s1, imm2: (
            (in0.astype(np.float32) + s0)
            * (in1.astype(np.float32) - in0.astype(np.float32) + s1)
        ),
    ),
)

# accum_out += sum(recip1(in0) * in1): seeded 1-Newton 1/x (rel err ~2e-3)
# times in1, with fused free-dim reduction.
_not_x = Bin(DAluOp.BITWISE_NOT, Src0, Src0)
_y0 = _not_x * C0


def _ref_recip1_mul_reduce(in0, in1, s0, s1, imm2):
    not_x = (~in0.view(np.int32)).view(np.float32)
    y0 = not_x * s0
    y1 = y0 * (s1 - in0 * y0)
    b = (y1 * in1).astype(np.float32)
    return b, b.reshape(b.shape[0], -1).sum(axis=-1, keepdims=True)


RECIP1_MUL_REDUCE = _register_dve_op(
    "RECIP1_MUL_REDUCE",
    Spec(
        body=(_y0 * (DC1 - Src0 * _y0)) * Src1,
        accum=_op_add,
        accum_init=Zero,
        reference=_ref_recip1_mul_reduce,
    ),
)


# ---------------------------------------------------------------- weights
def _gauss_taps() -> np.ndarray:
    x = np.arange(WS, dtype=np.float32) - (WS // 2)
    g = np.exp(-(x**2) / np.float32(2.0 * SIGMA * SIGMA)).astype(np.float32)
    return (g / g.sum()).astype(np.float32)


def _band(in0: int, k: int, o0: int, m: int, taps: np.ndarray) -> np.ndarray:
    """G[i, j] = taps[(in0+i) - (o0+j) + PAD]; zero outside the band."""
    gi = np.arange(k)[:, None] + in0
    gj = np.arange(m)[None, :] + o0
    d = gi - gj + PAD
    ok = (d >= 0) & (d < WS)
    out = np.zeros((k, m), np.float32)
    out[ok] = taps[d[ok]]
    return out


# packed weight layout: 4 variants x 512 cols of band matrices (bf16)
_VARS = ["g", "g2x2", "b1", "b2"]
_COL0 = [0, 118, 236, 354, 472]  # column offset of tile j within a variant
WPACK_COLS = 4 * 512


def _weight_arrays() -> dict[str, np.ndarray]:
    g = _gauss_taps()
    taps = {
        "g": g,
        "g2x2": (2.0 * g).astype(np.float32),
        "b1": np.ones(WS, np.float32),
        "b2": np.full(WS, np.float32(1.0 / (WS * WS)), np.float32),
    }
    wp = np.zeros((128, WPACK_COLS), np.float32)
    for vi, v in enumerate(_VARS):
        for j, (i0, k, o0, m) in enumerate(TILES):
            wp[:k, vi * 512 + _COL0[j] : vi * 512 + _COL0[j] + m] = _band(
                i0, k, o0, m, taps[v]
            )
    return {"wpack": wp.astype(BF16NP)}


# ---------------------------------------------------------------- bass build
def _build_bass() -> bass.Bass:
    nc = bacc.Bacc()

    img1_d = nc.declare_dram_parameter("img1", [BL, C, H, W], BF16, isOutput=False)
    img2_d = nc.declare_dram_parameter("img2", [BL, C, H, W], BF16, isOutput=False)
    match_d = nc.declare_dram_parameter("matchb", [BL, 1, H, W], BF16, isOutput=False)
    wpack_d = nc.declare_dram_parameter("wpack", [128, WPACK_COLS], BF16, isOutput=False)
    out_d = nc.declare_dram_parameter("out", [BL, 128, 2], F32, isOutput=True)

    from contextlib import ExitStack

    with TileContext(nc) as tc, ExitStack() as ctx:
        consts = ctx.enter_context(tc.tile_pool(name="consts", bufs=1))
        imgs = ctx.enter_context(tc.tile_pool(name="imgs", bufs=2))
        pre = ctx.enter_context(tc.tile_pool(name="pre", bufs=2))
        tsbp = ctx.enter_context(tc.tile_pool(name="tsb", bufs=3))
        pix = ctx.enter_context(tc.tile_pool(name="pix", bufs=2))
        maskp = ctx.enter_context(tc.tile_pool(name="maskp", bufs=1))
        accp = ctx.enter_context(tc.tile_pool(name="accp", bufs=1))
        psumT = ctx.enter_context(tc.tile_pool(name="psumT", bufs=1, space="PSUM"))
        psumF = ctx.enter_context(tc.tile_pool(name="psumF", bufs=1, space="PSUM"))

        wpack = consts.tile([128, WPACK_COLS], BF16, tag="wpack", name="wpack")
        nc.sync.dma_start(out=wpack, in_=wpack_d[:, :])

        def wsl(var: str, j: int):
            vi = _VARS.index(var)
            i0, k, o0, m = TILES[j]
            c0 = vi * 512 + _COL0[j]
            return wpack[:k, c0 : c0 + m]

        def dma_pack(dst, src_bchw):
            """DMA the 5 H-tiles of one [H,W] image plane into a [128, 2560]
            pack; tiles 0-3 load a full 128 rows (in bounds), tile 4 is 45."""
            for j, (i0, k, o0, m) in enumerate(TILES):
                rows = 128 if j < NT - 1 else k
                nc.sync.dma_start(
                    out=dst[:rows, j * 512 : j * 512 + 512],
                    in_=src_bchw[i0 : i0 + rows, :],
                )

        def conv_pass1(pack, var, wb, tdst, s):
            """vertical conv + transpose of field s into tdst cols
            [s*512, (s+1)*512): col range j holds out-rows o0:o0+m of H."""
            w0, kw, _, _ = TILES[wb]
            for j, (i0, k, o0, m) in enumerate(TILES):
                nc.tensor.matmul(
                    tdst[:kw, s * 512 + o0 : s * 512 + o0 + m],
                    pack[:k, j * 512 + w0 : j * 512 + w0 + kw],
                    wsl(var, j),
                    start=True,
                    stop=True,
                )

        for b in range(BL):
            scols = accp.tile([128, 16], F32, tag="scols", name=f"scols_{b}")
            mcols = accp.tile([128, NT], F32, tag="mcols", name=f"mcols_{b}")
            nc.vector.memset(scols, 0.0)
            nc.vector.memset(mcols, 0.0)

            # ---------------- mask pipeline (box conv of match) -------------
            mpack = imgs.tile([128, NT * 512], BF16, tag="mpack", name="mpack")
            dma_pack(mpack, match_d[b, 0])

            bmask = []
            for wb, (w0, kw, ow0, mw) in enumerate(TILES):
                tm = psumT.tile([128, 2048], F32, tag="T")
                conv_pass1(mpack, "b1", wb, tm, 0)
                tsb = tsbp.tile([128, 512], BF16, tag="tsb_m")
                nc.scalar.copy(tsb[:kw, :], tm[:kw, :512])
                fp = psumF.tile([128, 2048], F32, tag="F")
                nc.tensor.matmul(
                    fp[:mw, :512], wsl("b2", wb), tsb[:kw, :], start=True, stop=True
                )
                mk = maskp.tile([128, 512], BF16, tag=f"mask_{wb}", name=f"mask_{wb}")
                # bmask = (boxconv > 0.5) as 1.0/0.0; count rides the accum
                nc.vector.tensor_scalar(
                    mk[:mw, :], fp[:mw, :512], 0.5, 0.0,
                    ALU.is_gt, ALU.add,
                    accum_out=mcols[:mw, wb : wb + 1],
                )
                bmask.append(mk)

            # ---------------- channels -------------------------------------
            for c in range(C):
                x1p = imgs.tile([128, NT * 512], BF16, tag="x1p", name="x1p")
                x2p = imgs.tile([128, NT * 512], BF16, tag="x2p", name="x2p")
                dma_pack(x1p, img1_d[b, c])
                dma_pack(x2p, img2_d[b, c])

                # pre: p12 = x1*x2 (GPSIMD), ssq = x1^2+x2^2 (custom DVE);
                # tiles 0-3 fully DMA'd -> one FD-2048 op + tile-4 tail
                p12 = pre.tile([128, NT * 512], BF16, tag="p12", name="p12")
                ssq = pre.tile([128, NT * 512], BF16, tag="ssq", name="ssq")
                k4 = TILES[4][1]
                nc.gpsimd.tensor_mul(p12[:, :2048], x1p[:, :2048], x2p[:, :2048])
                nc.gpsimd.tensor_mul(
                    p12[:k4, 2048:], x1p[:k4, 2048:], x2p[:k4, 2048:]
                )
                nc.vector._custom_dve(
                    SQSUM_ANT, out=ssq[:, :2048],
                    in0=x1p[:, :2048], in1=x2p[:, :2048], s0=2.0,
                )
                nc.vector._custom_dve(
                    SQSUM_ANT, out=ssq[:k4, 2048:],
                    in0=x1p[:k4, 2048:], in1=x2p[:k4, 2048:], s0=2.0,
                )

                for gi, grp in enumerate(WB_GROUPS):
                    gmw = TILES[grp[0]][3]
                    gn = len(grp)
                    nump = pix.tile([128, 1024], BF16, tag="nump")
                    denp = pix.tile([128, 1024], F32, tag="denp")
                    selp = pix.tile([128, 1024], BF16, tag="selp")

                    for wi, wb in enumerate(grp):
                        w0, kw, ow0, mw = TILES[wb]
                        t4 = psumT.tile([128, 2048], F32, tag="T")
                        conv_pass1(x1p, "g", wb, t4, 0)
                        conv_pass1(x2p, "g", wb, t4, 1)
                        conv_pass1(p12, "g", wb, t4, 2)
                        conv_pass1(ssq, "g", wb, t4, 3)

                        tsb = tsbp.tile([128, 2048], BF16, tag="tsb")
                        if wb % 3 == 2:
                            nc.vector.tensor_copy(tsb[:kw, :], t4[:kw, :])
                        else:
                            nc.scalar.copy(tsb[:kw, :], t4[:kw, :])

                        f4 = psumF.tile([128, 2048], F32, tag="F")
                        nc.tensor.matmul(f4[:mw, 0:512], wsl("g", wb),
                                         tsb[:kw, 0:512], start=True, stop=True)
                        nc.tensor.matmul(f4[:mw, 512:1024], wsl("g", wb),
                                         tsb[:kw, 512:1024], start=True, stop=True)
                        nc.tensor.matmul(f4[:mw, 1024:1536], wsl("g2x2", wb),
                                         tsb[:kw, 1024:1536], start=True, stop=True)
                        nc.tensor.matmul(f4[:mw, 1536:2048], wsl("g", wb),
                                         tsb[:kw, 1536:2048], start=True, stop=True)

                        # ---- fused per-pixel SSIM --------------------------
                        e12 = pix.tile([128, 1024], BF16, tag="e12")
                        nc.scalar.activation(
                            e12[:mw, :], f4[:mw, 0:1024], AF.Copy, scale=SQRT2
                        )
                        e1 = e12[:mw, 0:512]
                        e2 = e12[:mw, 512:1024]

                        P2 = pix.tile([128, 512], BF16, tag="P2")
                        Ut = pix.tile([128, 512], BF16, tag="Ut")
                        nc.gpsimd.tensor_mul(P2[:mw, :], e1, e2)
                        nc.gpsimd.tensor_add(Ut[:mw, :], e1, e2)
                        U2 = pix.tile([128, 512], BF16, tag="U2")
                        nc.scalar.activation(
                            U2[:mw, :], Ut[:mw, :], AF.Square, scale=ISQRT2
                        )
                        Q2 = pix.tile([128, 512], BF16, tag="Q2")
                        nc.vector.tensor_sub(Q2[:mw, :], U2[:mw, :], P2[:mw, :])

                        nc.vector._custom_dve(
                            FMA2_ANT, out=nump[:mw, wi * 512 : wi * 512 + 512],
                            in0=P2[:mw, :], in1=f4[:mw, 1024:1536],
                            s0=SSIM_C1, s1=SSIM_C2,
                        )
                        nc.vector._custom_dve(
                            FMA2_ANT, out=denp[:mw, wi * 512 : wi * 512 + 512],
                            in0=Q2[:mw, :], in1=f4[:mw, 1536:2048],
                            s0=SSIM_C1, s1=SSIM_C2,
                        )
                        nc.vector.tensor_mul(
                            selp[:mw, wi * 512 : wi * 512 + 512],
                            nump[:mw, wi * 512 : wi * 512 + 512],
                            bmask[wb][:mw, :],
                        )

                    # masked ssim sum over the whole group in one fused op
                    junk = pix.tile([128, 1024], BF16, tag="junk")
                    nc.vector._custom_dve(
                        RECIP1_MUL_REDUCE,
                        out=junk[:gmw, : gn * 512],
                        in0=denp[:gmw, : gn * 512],
                        in1=selp[:gmw, : gn * 512],
                        s0=RECIP_S0, s1=RECIP_S1,
                        accum_out=scols[:gmw, c * NG + gi : c * NG + gi + 1],
                    )

            # ---------------- per-image reduction --------------------------
            fin = accp.tile([128, 2], F32, tag="fin", name=f"fin_{b}")
            nc.vector.tensor_reduce(
                fin[:, 0:1], scols[:, : C * NG], mybir.AxisListType.X, ALU.add
            )
            nc.vector.tensor_reduce(
                fin[:, 1:2], mcols[:, :], mybir.AxisListType.X, ALU.add
            )
            nc.sync.dma_start(out=out_d[b], in_=fin[:, :2])

    nc.compile()
    return nc


_NC_CACHE: bass.Bass | None = None


def _get_nc() -> bass.Bass:
    global _NC_CACHE
    if _NC_CACHE is None:
        _NC_CACHE = _build_bass()
    return _NC_CACHE


def _make_in_maps(img1: np.ndarray, img2: np.ndarray, match: np.ndarray):
    img1b = np.ascontiguousarray(img1.astype(BF16NP))
    img2b = np.ascontiguousarray(img2.astype(BF16NP))
    matchb = np.ascontiguousarray(match.astype(BF16NP))

    weights = _weight_arrays()
    in_maps = []
    for i in range(NCORES):
        sl = slice(i * BL, (i + 1) * BL)
        m = {"img1": img1b[sl], "img2": img2b[sl], "matchb": matchb[sl]}
        m.update(weights)
        in_maps.append(m)
    return in_maps


def kernel(img1: np.ndarray, img2: np.ndarray, match: np.ndarray) -> np.ndarray:
    in_maps = _make_in_maps(img1, img2, match)
    nc = _get_nc()
    res = run_bass_kernel_spmd(nc, in_maps, list(range(NCORES))).results

    total = np.float64(0.0)
    for i in range(NCORES):
        o = np.asarray(res[i]["out"], dtype=np.float64)  # [BL, 128, 2]
        s1 = o[:, :, 0].sum(axis=1)  # sum(mask * ssim) per image
        cnt = o[:, :, 1].sum(axis=1)  # mask pixel count per image
        m_b = cnt + 1e-7 * (H * W)  # reference adds 1e-7 to every mask px
        s_b = 3.0 * m_b - s1
        norm = (H * W) / (m_b + 1e-6)
        total += np.sum(s_b * norm)
    return np.float32(total / 3.0)


# revision 16
# speedup vs baseline: 1.6873x; 1.2640x over previous
"""SSIM masked loss kernel for Trainium2 (8 NeuronCores, data-parallel over batch).

v3 — bf16 end-to-end, packed PSUM/SBUF tiles, fused per-pixel math.

Strategy
--------
Pure data parallel: B=16 images sharded 2 per core.  Inputs are converted to
bf16 on the host (halves HBM traffic; the final loss is a 12.6M-pixel sum of
(1-ssim)*mask with ssim ~ 0.005, so per-pixel rounding noise is damped ~200x
in the result).

The separable 11-tap Gaussian window runs on the TensorEngine as two banded
matmuls per tile (pass 1: image-stationary, output lands transposed; pass 2:
band-stationary, N=512 streams), all in bf16.

Packing: the four pass-1 results share one 4-bank PSUM tile so a single
copy evacuates all of them; pass-2 puts mu1|mu2 in adjacent banks so one
ACT copy produces both scaled means.  Per-pixel math (per [mw,512] tile):

  e12  = sqrt2 * (mu1|mu2)                 (1 ACT copy, PSUM->SBUF bf16)
  P2   = e1*e2      = 2*mu1*mu2            (GPSIMD)
  U    = e1+e2                             (GPSIMD)
  U2   = (U/sqrt2)^2 = (mu1+mu2)^2         (ACT square)
  Q2   = U2 - P2    = mu1^2+mu2^2          (DVE tensor_tensor, 2x bf16)
  num  = (P2+C1)*(R-P2+C2) = num1*num2     (custom DVE; R = 2*E[x1x2])
  den  = (Q2+C1)*(S-Q2+C2) = den1*den2     (custom DVE; S = E[x1^2+x2^2])
  sel  = num * bmask                       (DVE tensor_tensor, 2x bf16)
  acc += sum(sel * recip1(den))            (custom DVE: seeded 1-Newton
                                            reciprocal * sel, fused reduce)

The mask is thresholded once per (b,wb) with a fused compare+count op.  The
reference's +1e-7 mask epsilons contribute ~2e-7 relative and are applied on
the host; per-image partials leave the device as [128,2] per-partition sums.
"""

import sys

import numpy as np

sys.path.insert(0, "/opt/trn_rl_repo")

import ml_dtypes  # noqa: E402

import concourse.bass as bass  # noqa: E402
import concourse.bacc as bacc  # noqa: E402
from concourse import mybir  # noqa: E402
from concourse.bass_utils import run_bass_kernel_spmd  # noqa: E402
from concourse.tile import TileContext  # noqa: E402

import concourse.dve_ops as dve_ops_mod  # noqa: E402
from concourse.dve_ops import DveOp  # noqa: E402
from concourse.dve_spec import (  # noqa: E402
    C0,
    C1 as DC1,
    AluOp as DAluOp,
    Bin,
    Spec,
    Src0,
    Src1,
    Zero,
    _has_src1,
    lower as dve_lower,
)
from concourse.dve_uop import DveOpSpec  # noqa: E402
from operator import add as _op_add  # noqa: E402

WS = 11
PAD = WS // 2
SIGMA = 1.5
SSIM_C1 = float(0.01**2)
SSIM_C2 = float(0.03**2)
SQRT2 = float(np.sqrt(2.0))
ISQRT2 = float(1.0 / np.sqrt(2.0))
# Chebyshev seed pair for the BITWISE_NOT reciprocal approximation
RECIP_S0 = -0.23549792
RECIP_S1 = 2.0017324

B, C, H, W = 16, 3, 512, 512
NCORES = 8
BL = B // NCORES  # images per core

# output tiles of 118 rows; input windows of <=128 rows (clipped at edges)
# (in_start, in_size, out_start, out_size)
TILES = [
    (0, 123, 0, 118),
    (113, 128, 118, 118),
    (231, 128, 236, 118),
    (349, 128, 354, 118),
    (467, 45, 472, 40),
]
NT = len(TILES)
# per-pixel stage groups of W-windows with equal mw, processed as one op
WB_GROUPS = [(0, 1), (2, 3), (4,)]
NG = len(WB_GROUPS)

F32 = mybir.dt.float32
BF16 = mybir.dt.bfloat16
AF = mybir.ActivationFunctionType
ALU = mybir.AluOpType
BF16NP = ml_dtypes.bfloat16


# ---------------------------------------------------------------- custom DVE
def _register_dve_op(name: str, spec: Spec) -> DveOp:
    if name in dve_ops_mod._SUB_OPCODE_FOR_NAME:
        return next(op for op in dve_ops_mod.OPS if op.name == name)
    row = max(dve_ops_mod._SUB_OPCODE_FOR_NAME.values()) + 1
    assert row < 0x20, "out of custom-DVE opcode rows"
    dve_ops_mod._SUB_OPCODE_FOR_NAME[name] = row
    shas = {}
    for ver in ("v3", "v4"):
        uops = dve_lower(spec, ver=ver)
        shas[ver] = DveOpSpec(
            name=name, opcode=row, uops=uops, rd1_en=_has_src1(spec)
        ).sha(ver)
    op = DveOp(name, spec, subdim=False, uops_sha=shas)
    dve_ops_mod.OPS.append(op)
    dve_ops_mod.CUSTOM_DVE_SPECS[name] = spec
    return op


# ssq = (x1-x2)^2 + s0*x1*x2; s0=2 gives x1^2 + x2^2 (pre-conv input prep).
# (sq(Src0)+sq(Src1) hard-crashes the exec unit; only pipeline values can
# be squared, so use the (a-b)^2+2ab form.)
SQSUM_ANT = _register_dve_op(
    "SQSUM_ANT",
    Spec(
        body=(Src0 - Src1) * (Src0 - Src1) + Src0 * Src1 * C0,
        reference=lambda in0, in1, s0, s1, imm2: (
            (in0.astype(np.float32) - in1.astype(np.float32)) ** 2
            + in0.astype(np.float32) * in1.astype(np.float32) * s0
        ),
    ),
)

# Q2 = s0*(e1-e2)^2 + e1*e2; with e = sqrt2*mu and s0=0.5: mu1^2 + mu2^2
SQMA_ANT = _register_dve_op(
    "SQMA_ANT",
    Spec(
        body=(Src0 - Src1) * (Src0 - Src1) * C0 + Src0 * Src1,
        reference=lambda in0, in1, s0, s1, imm2: (
            (in0.astype(np.float32) - in1.astype(np.float32)) ** 2 * s0
            + in0.astype(np.float32) * in1.astype(np.float32)
        ),
    ),
)

# out = (in0 + s0) * (in1 - in0 + s1): num1*num2 / den1*den2 in one op
FMA2_ANT = _register_dve_op(
    "FMA2_ANT",
    Spec(
        body=(Src0 + C0) * (Src1 - Src0 + DC1),
        reference=lambda in0, in1, s0, s1, imm2: (
            (in0.astype(np.float32) + s0)
            * (in1.astype(np.float32) - in0.astype(np.float32) + s1)
        ),
    ),
)

# accum_out += sum(recip1(in0) * in1): seeded 1-Newton 1/x (rel err ~2e-3)
# times in1, with fused free-dim reduction.
_not_x = Bin(DAluOp.BITWISE_NOT, Src0, Src0)
_y0 = _not_x * C0


def _ref_recip1_mul_reduce(in0, in1, s0, s1, imm2):
    not_x = (~in0.view(np.int32)).view(np.float32)
    y0 = not_x * s0
    y1 = y0 * (s1 - in0 * y0)
    b = (y1 * in1).astype(np.float32)
    return b, b.reshape(b.shape[0], -1).sum(axis=-1, keepdims=True)


RECIP1_MUL_REDUCE = _register_dve_op(
    "RECIP1_MUL_REDUCE",
    Spec(
        body=(_y0 * (DC1 - Src0 * _y0)) * Src1,
        accum=_op_add,
        accum_init=Zero,
        reference=_ref_recip1_mul_reduce,
    ),
)


# ---------------------------------------------------------------- weights
def _gauss_taps() -> np.ndarray:
    x = np.arange(WS, dtype=np.float32) - (WS // 2)
    g = np.exp(-(x**2) / np.float32(2.0 * SIGMA * SIGMA)).astype(np.float32)
    return (g / g.sum()).astype(np.float32)


def _band(in0: int, k: int, o0: int, m: int, taps: np.ndarray) -> np.ndarray:
    """G[i, j] = taps[(in0+i) - (o0+j) + PAD]; zero outside the band."""
    gi = np.arange(k)[:, None] + in0
    gj = np.arange(m)[None, :] + o0
    d = gi - gj + PAD
    ok = (d >= 0) & (d < WS)
    out = np.zeros((k, m), np.float32)
    out[ok] = taps[d[ok]]
    return out


# packed weight layout: 5 variants x 512 cols of band matrices (bf16)
_VARS = ["g", "g2x2", "b1", "b2", "gs2"]
_COL0 = [0, 118, 236, 354, 472]  # column offset of tile j within a variant
WPACK_COLS = 5 * 512


def _weight_arrays() -> dict[str, np.ndarray]:
    g = _gauss_taps()
    taps = {
        "g": g,
        "g2x2": (2.0 * g).astype(np.float32),
        "b1": np.ones(WS, np.float32),
        "b2": np.full(WS, np.float32(1.0 / (WS * WS)), np.float32),
        "gs2": (np.sqrt(np.float32(2.0)) * g).astype(np.float32),
    }
    wp = np.zeros((128, WPACK_COLS), np.float32)
    for vi, v in enumerate(_VARS):
        for j, (i0, k, o0, m) in enumerate(TILES):
            wp[:k, vi * 512 + _COL0[j] : vi * 512 + _COL0[j] + m] = _band(
                i0, k, o0, m, taps[v]
            )
    return {"wpack": wp.astype(BF16NP)}


# ---------------------------------------------------------------- bass build
def _build_bass() -> bass.Bass:
    nc = bacc.Bacc()

    img1_d = nc.declare_dram_parameter("img1", [BL, C, H, W], BF16, isOutput=False)
    img2_d = nc.declare_dram_parameter("img2", [BL, C, H, W], BF16, isOutput=False)
    match_d = nc.declare_dram_parameter("matchb", [BL, 1, H, W], BF16, isOutput=False)
    wpack_d = nc.declare_dram_parameter("wpack", [128, WPACK_COLS], BF16, isOutput=False)
    out_d = nc.declare_dram_parameter("out", [BL, 128, 2], F32, isOutput=True)

    from contextlib import ExitStack

    with TileContext(nc) as tc, ExitStack() as ctx:
        consts = ctx.enter_context(tc.tile_pool(name="consts", bufs=1))
        imgs = ctx.enter_context(tc.tile_pool(name="imgs", bufs=2))
        pre = ctx.enter_context(tc.tile_pool(name="pre", bufs=2))
        tsbp = ctx.enter_context(tc.tile_pool(name="tsb", bufs=3))
        pix = ctx.enter_context(tc.tile_pool(name="pix", bufs=3))
        maskp = ctx.enter_context(tc.tile_pool(name="maskp", bufs=1))
        accp = ctx.enter_context(tc.tile_pool(name="accp", bufs=1))
        psumT = ctx.enter_context(tc.tile_pool(name="psumT", bufs=1, space="PSUM"))
        psumF = ctx.enter_context(tc.tile_pool(name="psumF", bufs=1, space="PSUM"))

        wpack = consts.tile([128, WPACK_COLS], BF16, tag="wpack", name="wpack")
        nc.sync.dma_start(out=wpack, in_=wpack_d[:, :])

        def wsl(var: str, j: int):
            vi = _VARS.index(var)
            i0, k, o0, m = TILES[j]
            c0 = vi * 512 + _COL0[j]
            return wpack[:k, c0 : c0 + m]

        def dma_pack(dst, src_bchw):
            """DMA the 5 H-tiles of one [H,W] image plane into a [128, 2560]
            pack; tiles 0-3 load a full 128 rows (in bounds), tile 4 is 45."""
            for j, (i0, k, o0, m) in enumerate(TILES):
                rows = 128 if j < NT - 1 else k
                nc.sync.dma_start(
                    out=dst[:rows, j * 512 : j * 512 + 512],
                    in_=src_bchw[i0 : i0 + rows, :],
                )

        def conv_pass1(pack, var, wb, tdst, s):
            """vertical conv + transpose of field s into tdst cols
            [s*512, (s+1)*512): col range j holds out-rows o0:o0+m of H."""
            w0, kw, _, _ = TILES[wb]
            for j, (i0, k, o0, m) in enumerate(TILES):
                nc.tensor.matmul(
                    tdst[:kw, s * 512 + o0 : s * 512 + o0 + m],
                    pack[:k, j * 512 + w0 : j * 512 + w0 + kw],
                    wsl(var, j),
                    start=True,
                    stop=True,
                )

        for b in range(BL):
            scols = accp.tile([128, 16], F32, tag="scols", name=f"scols_{b}")
            mcols = accp.tile([128, NT], F32, tag="mcols", name=f"mcols_{b}")
            nc.vector.memset(scols, 0.0)
            nc.vector.memset(mcols, 0.0)

            # ---------------- mask pipeline (box conv of match) -------------
            mpack = imgs.tile([128, NT * 512], BF16, tag="mpack", name="mpack")
            dma_pack(mpack, match_d[b, 0])

            bmask = []
            for wb, (w0, kw, ow0, mw) in enumerate(TILES):
                tm = psumT.tile([128, 2048], F32, tag="T")
                conv_pass1(mpack, "b1", wb, tm, 0)
                tsb = tsbp.tile([128, 512], BF16, tag="tsb_m")
                nc.scalar.copy(tsb[:kw, :], tm[:kw, :512])
                fp = psumF.tile([128, 2048], F32, tag="F")
                nc.tensor.matmul(
                    fp[:mw, :512], wsl("b2", wb), tsb[:kw, :], start=True, stop=True
                )
                mk = maskp.tile([128, 512], BF16, tag=f"mask_{wb}", name=f"mask_{wb}")
                # bmask = (boxconv > 0.5) as 1.0/0.0; count rides the accum
                nc.vector.tensor_scalar(
                    mk[:mw, :], fp[:mw, :512], 0.5, 0.0,
                    ALU.is_gt, ALU.add,
                    accum_out=mcols[:mw, wb : wb + 1],
                )
                bmask.append(mk)

            # ---------------- channels -------------------------------------
            for c in range(C):
                x1p = imgs.tile([128, NT * 512], BF16, tag="x1p", name="x1p")
                x2p = imgs.tile([128, NT * 512], BF16, tag="x2p", name="x2p")
                dma_pack(x1p, img1_d[b, c])
                dma_pack(x2p, img2_d[b, c])

                # pre: p12 = x1*x2 (GPSIMD), ssq = x1^2+x2^2 (custom DVE);
                # tiles 0-3 fully DMA'd -> one FD-2048 op + tile-4 tail
                p12 = pre.tile([128, NT * 512], BF16, tag="p12", name="p12")
                ssq = pre.tile([128, NT * 512], BF16, tag="ssq", name="ssq")
                k4 = TILES[4][1]
                nc.gpsimd.tensor_mul(p12[:, :2048], x1p[:, :2048], x2p[:, :2048])
                nc.gpsimd.tensor_mul(
                    p12[:k4, 2048:], x1p[:k4, 2048:], x2p[:k4, 2048:]
                )
                nc.vector._custom_dve(
                    SQSUM_ANT, out=ssq[:, :2048],
                    in0=x1p[:, :2048], in1=x2p[:, :2048], s0=2.0,
                )
                nc.vector._custom_dve(
                    SQSUM_ANT, out=ssq[:k4, 2048:],
                    in0=x1p[:k4, 2048:], in1=x2p[:k4, 2048:], s0=2.0,
                )

                for gi, grp in enumerate(WB_GROUPS):
                    gmw = TILES[grp[0]][3]
                    gn = len(grp)
                    nump = pix.tile([128, 1024], BF16, tag="nump")
                    denp = pix.tile([128, 1024], F32, tag="denp")
                    selp = pix.tile([128, 1024], BF16, tag="selp")

                    for wi, wb in enumerate(grp):
                        w0, kw, ow0, mw = TILES[wb]
                        t4 = psumT.tile([128, 2048], F32, tag="T")
                        conv_pass1(x1p, "g", wb, t4, 0)
                        conv_pass1(x2p, "g", wb, t4, 1)
                        conv_pass1(p12, "g", wb, t4, 2)
                        conv_pass1(ssq, "g", wb, t4, 3)

                        tsb = tsbp.tile([128, 2048], BF16, tag="tsb")
                        nc.scalar.copy(tsb[:kw, :], t4[:kw, :])

                        # pass2: [sqrt2*mu1 | sqrt2*mu2 | R | S] in 4 banks
                        f4 = psumF.tile([128, 2048], F32, tag="F")
                        nc.tensor.matmul(f4[:mw, 0:512], wsl("gs2", wb),
                                         tsb[:kw, 0:512], start=True, stop=True)
                        nc.tensor.matmul(f4[:mw, 512:1024], wsl("gs2", wb),
                                         tsb[:kw, 512:1024], start=True, stop=True)
                        nc.tensor.matmul(f4[:mw, 1024:1536], wsl("g2x2", wb),
                                         tsb[:kw, 1024:1536], start=True, stop=True)
                        nc.tensor.matmul(f4[:mw, 1536:2048], wsl("g", wb),
                                         tsb[:kw, 1536:2048], start=True, stop=True)

                        # single evac frees all four F banks at once
                        f4sb = pix.tile([128, 2048], BF16, tag="f4sb")
                        nc.scalar.copy(f4sb[:mw, :], f4[:mw, :])
                        e1 = f4sb[:mw, 0:512]
                        e2 = f4sb[:mw, 512:1024]

                        # ---- fused per-pixel SSIM (all SBUF-side) ----------
                        P2 = pix.tile([128, 512], BF16, tag="P2")
                        nc.gpsimd.tensor_mul(P2[:mw, :], e1, e2)
                        Q2 = pix.tile([128, 512], BF16, tag="Q2")
                        nc.vector._custom_dve(
                            SQMA_ANT, out=Q2[:mw, :], in0=e1, in1=e2, s0=0.5
                        )

                        nc.vector._custom_dve(
                            FMA2_ANT, out=nump[:mw, wi * 512 : wi * 512 + 512],
                            in0=P2[:mw, :], in1=f4sb[:mw, 1024:1536],
                            s0=SSIM_C1, s1=SSIM_C2,
                        )
                        nc.vector._custom_dve(
                            FMA2_ANT, out=denp[:mw, wi * 512 : wi * 512 + 512],
                            in0=Q2[:mw, :], in1=f4sb[:mw, 1536:2048],
                            s0=SSIM_C1, s1=SSIM_C2,
                        )
                        nc.vector.tensor_mul(
                            selp[:mw, wi * 512 : wi * 512 + 512],
                            nump[:mw, wi * 512 : wi * 512 + 512],
                            bmask[wb][:mw, :],
                        )

                    # masked ssim sum over the whole group in one fused op
                    junk = pix.tile([128, 1024], BF16, tag="junk")
                    nc.vector._custom_dve(
                        RECIP1_MUL_REDUCE,
                        out=junk[:gmw, : gn * 512],
                        in0=denp[:gmw, : gn * 512],
                        in1=selp[:gmw, : gn * 512],
                        s0=RECIP_S0, s1=RECIP_S1,
                        accum_out=scols[:gmw, c * NG + gi : c * NG + gi + 1],
                    )

            # ---------------- per-image reduction --------------------------
            fin = accp.tile([128, 2], F32, tag="fin", name=f"fin_{b}")
            nc.vector.tensor_reduce(
                fin[:, 0:1], scols[:, : C * NG], mybir.AxisListType.X, ALU.add
            )
            nc.vector.tensor_reduce(
                fin[:, 1:2], mcols[:, :], mybir.AxisListType.X, ALU.add
            )
            nc.sync.dma_start(out=out_d[b], in_=fin[:, :2])

    nc.compile()
    return nc


_NC_CACHE: bass.Bass | None = None


def _get_nc() -> bass.Bass:
    global _NC_CACHE
    if _NC_CACHE is None:
        _NC_CACHE = _build_bass()
    return _NC_CACHE


def _make_in_maps(img1: np.ndarray, img2: np.ndarray, match: np.ndarray):
    img1b = np.ascontiguousarray(img1.astype(BF16NP))
    img2b = np.ascontiguousarray(img2.astype(BF16NP))
    matchb = np.ascontiguousarray(match.astype(BF16NP))

    weights = _weight_arrays()
    in_maps = []
    for i in range(NCORES):
        sl = slice(i * BL, (i + 1) * BL)
        m = {"img1": img1b[sl], "img2": img2b[sl], "matchb": matchb[sl]}
        m.update(weights)
        in_maps.append(m)
    return in_maps


def kernel(img1: np.ndarray, img2: np.ndarray, match: np.ndarray) -> np.ndarray:
    in_maps = _make_in_maps(img1, img2, match)
    nc = _get_nc()
    res = run_bass_kernel_spmd(nc, in_maps, list(range(NCORES))).results

    total = np.float64(0.0)
    for i in range(NCORES):
        o = np.asarray(res[i]["out"], dtype=np.float64)  # [BL, 128, 2]
        s1 = o[:, :, 0].sum(axis=1)  # sum(mask * ssim) per image
        cnt = o[:, :, 1].sum(axis=1)  # mask pixel count per image
        m_b = cnt + 1e-7 * (H * W)  # reference adds 1e-7 to every mask px
        s_b = 3.0 * m_b - s1
        norm = (H * W) / (m_b + 1e-6)
        total += np.sum(s_b * norm)
    return np.float32(total / 3.0)
